# revision 47
# baseline (speedup 1.0000x reference)
"""Trainium2 Bass kernel for DockingAwareAttention (B=2, S=2048, D=1024, H=16).

Reference:  attn = (1-beta)*softmax(Q K^T / 8) + beta * ds[None, :]
            out  = attn @ V @ Wo + bo

Sharding (8 NeuronCores): data-parallel over batch (cores 0-3 <-> b=0,
4-7 <-> b=1) x tensor-parallel over heads (4 heads = 256 head-dims per
core; Q/K/V column-sharded, Wo row-sharded).  Each core writes a full
(S, D) partial; the host sums the 4 partials per batch (the TP
"all-reduce" of a row-sharded Wo), applies (1-beta), and adds bo.

Device-side structure (per core, one SPMD program):
  - Transposed dataflow: Q^T/K^T (head-dim on partitions) come straight
    out of the projection matmuls; scores are computed as S^T = K Q^T,
    exp'd on ScalarE into bf16 P^T, which feeds ctx^T = V^T P^T, which
    is exactly the lhsT of the output projection.  No transposes.
  - Softmax normalization is deferred: V carries a ones column, so each
    ctx matmul also produces the softmax row-sums (psum row 64); ctx is
    scaled by 1/rowsum afterwards (reciprocal + GPSIMD partition
    broadcast), off the critical path.
  - The docking blend is rank-1 in the query index and collapses to
    dock_h = ((x^T ds) Wv)_h -- computed on the host and added as a
    per-partition constant to ctx^T.
  - Score matmuls are row-packed: the two heads of a 128-partition
    chunk process the same 512-query half via two back-to-back 64-deep
    matmuls on PE row-strips 0-63/64-127 (tile_position auto-derived),
    which execute concurrently on the 16x 32x32 sub-array grid; one
    N=1024 ACTIVATE exps both heads' scores.
  - Deferred work (V projection, second Q/K chunk, output projection)
    is woven into the ACT-bound attention spans: producers are emitted
    inside the consuming key-loops (per-iteration hooks / filler
    queues), consumers are emitted late at low priority so the Tile
    list scheduler pulls them into PE stalls.
  - dtypes: bf16 activations/weights/P^T (fp32 matmul is 4 cycles/row
    on TRN2; bf16 is 1), fp32 PSUM accumulation and output partials.
"""

import os
import sys

for _p in ("/opt/trn_rl_repo", "/root/.axon_site/_ro/trn_rl_repo"):
    if os.path.isdir(_p) and _p not in sys.path:
        sys.path.append(_p)

import ml_dtypes
import numpy as np

# Problem shape (hardcoded per contest rules).
B, S, D, H = 2, 2048, 1024, 16
HD = 64          # head dim
NCORES = 8
GROUPS = NCORES // B      # 4 head-groups per batch
HPC = H // GROUPS         # 4 heads per core
DHC = HPC * HD            # 256 head-dims per core
P = 128


def build_module(s=S, d=D, qchunk=1024):
    """Build the per-core Bass module (same program on all 8 cores)."""
    import concourse.mybir as mybir
    import concourse.tile as tile
    from concourse import bacc

    f32 = mybir.dt.float32
    bf16 = mybir.dt.bfloat16
    AF = mybir.ActivationFunctionType
    ALU = mybir.AluOpType

    DC = d // P               # contraction chunks over model dim
    KC = s // P               # key tiles
    ST = s // P               # seq tiles
    qchunk = min(qchunk, s)
    NQC = s // qchunk         # query chunks per head
    NW = min(512, qchunk)     # matmul free-dim tile (one PSUM bank of f32)

    nc = bacc.Bacc("TRN2", target_bir_lowering=False, debug=False,
                   num_devices=NCORES)

    # ---- DRAM I/O (per core) ----
    xT_d = nc.dram_tensor("xT", [d, s], bf16, kind="ExternalInput")
    wq_d = nc.dram_tensor("wq", [d, DHC], bf16, kind="ExternalInput")
    wk_d = nc.dram_tensor("wk", [d, DHC], bf16, kind="ExternalInput")
    wv_d = nc.dram_tensor("wv", [d, DHC], bf16, kind="ExternalInput")
    wo_d = nc.dram_tensor("wo", [DHC, d], bf16, kind="ExternalInput")
    bq_d = nc.dram_tensor("bq", [DHC], f32, kind="ExternalInput")
    bk_d = nc.dram_tensor("bk", [DHC], f32, kind="ExternalInput")
    bv_d = nc.dram_tensor("bv", [DHC], f32, kind="ExternalInput")
    dock_d = nc.dram_tensor("dock", [DHC], f32, kind="ExternalInput")
    part_d = nc.dram_tensor("part", [s, d], f32, kind="ExternalOutput")

    with tile.TileContext(nc) as tc:
        with tc.tile_pool(name="persist", bufs=1) as persist:
            # ---- persistent SBUF tensors ----
            xT_sb = [persist.tile([P, s], bf16, name=f"xT{k}") for k in range(DC)]
            wq_sb = [persist.tile([P, DHC], bf16, name=f"wq{k}") for k in range(DC)]
            wk_sb = [persist.tile([P, DHC], bf16, name=f"wk{k}") for k in range(DC)]
            wv_sb = [persist.tile([P, DHC], bf16, name=f"wv{k}") for k in range(DC)]
            # Wo stored by head PAIR: rows = the pair's 128 head-dims
            wop_sb = [persist.tile([P, d], bf16, name=f"wop{p}")
                      for p in range(HPC // 2)]
            qt_sb = [persist.tile([P, s], bf16, name=f"qt{m}") for m in range(DHC // P)]
            kt_sb = [persist.tile([P, s], bf16, name=f"kt{m}") for m in range(DHC // P)]
            # V augmented with a ones column per head: [V_h | 1], so the
            # softmax row-sum rides along as psum row 64 of the ctx matmul.
            # ctx is stored by head PAIR (even head rows 0-63, odd head rows
            # 64-127, via a small DMA partition shift) so the output
            # projection contracts both heads in one 128-deep matmul.
            va_sb = [persist.tile([P, HPC * (HD + 1)], bf16, name=f"va{k}")
                     for k in range(KC)]
            ctxp_sb = [persist.tile([P, s], bf16, name=f"ctxp{p}")
                       for p in range(HPC // 2)]
            bq_sb = persist.tile([P, DHC // P], f32, name="bq_sb")
            bk_sb = persist.tile([P, DHC // P], f32, name="bk_sb")
            bv_bc = persist.tile([P, DHC], f32, name="bv_bc")
            dock_sb = persist.tile([HD, HPC], f32, name="dock_sb")

            # ---- loads (x chunks first: they gate the projections) ----
            for k in range(DC):
                nc.sync.dma_start(xT_sb[k][:], xT_d[k * P:(k + 1) * P, :])
                nc.sync.dma_start(wq_sb[k][:], wq_d[k * P:(k + 1) * P, :])
                nc.sync.dma_start(wk_sb[k][:], wk_d[k * P:(k + 1) * P, :])
                nc.sync.dma_start(wv_sb[k][:], wv_d[k * P:(k + 1) * P, :])
            for p in range(HPC // 2):
                nc.sync.dma_start(wop_sb[p][:], wo_d[p * P:(p + 1) * P, :])
            nc.sync.dma_start(bq_sb[:], bq_d[:].rearrange("(o p) -> p o", p=P))
            nc.sync.dma_start(bk_sb[:], bk_d[:].rearrange("(o p) -> p o", p=P))
            nc.sync.dma_start(bv_bc[:], bv_d[None, :].to_broadcast((P, DHC)))
            nc.sync.dma_start(dock_sb[:],
                              dock_d[:].rearrange("(h d) -> d h", d=HD))
            for k in range(KC):
                for h in range(HPC):
                    off = h * (HD + 1) + HD
                    nc.vector.memset(va_sb[k][:, off:off + 1], 1.0)

            # ---- projections (part 1): Q/K heads 0-1 (m=0), V ----
            # m=0 runs k-outer with all four n-tile accumulators live so the
            # matmuls chase the arriving xT DMA chunks instead of waiting for
            # the full activation load.
            with tc.tile_pool(name="psum_m0", bufs=1, space="PSUM") as pm0:
                pqt = [pm0.tile([P, NW], f32, name=f"pq{n}")
                       for n in range(s // NW)]
                pkt = [pm0.tile([P, NW], f32, name=f"pk{n}")
                       for n in range(s // NW)]
                for k in range(DC):
                    for n in range(s // NW):
                        nc.tensor.matmul(
                            pqt[n][:], lhsT=wq_sb[k][:, 0:P],
                            rhs=xT_sb[k][:, n * NW:(n + 1) * NW],
                            start=(k == 0), stop=(k == DC - 1))
                        nc.tensor.matmul(
                            pkt[n][:], lhsT=wk_sb[k][:, 0:P],
                            rhs=xT_sb[k][:, n * NW:(n + 1) * NW],
                            start=(k == 0), stop=(k == DC - 1))
                for n in range(s // NW):
                    nc.vector.tensor_scalar_add(
                        qt_sb[0][:, n * NW:(n + 1) * NW], pqt[n][:],
                        bq_sb[:, 0:1])
                    nc.vector.tensor_scalar_add(
                        kt_sb[0][:, n * NW:(n + 1) * NW], pkt[n][:],
                        bk_sb[:, 0:1])

            # ---- attention + deferred work (Q/K m=1 proj, O-proj) ----
            # The PE stream is ACT(softmax)-bound; filler matmuls (the second
            # Q/K projection chunk and the output projection) are drip-fed one
            # or two per key tile into the attention loops to fill PE slack.
            with tc.tile_pool(name="psum_s", bufs=2, space="PSUM") as ps_pool, \
                 tc.tile_pool(name="psum_ctx", bufs=1, space="PSUM") as pc_pool, \
                 tc.tile_pool(name="psum_defer", bufs=1, space="PSUM") as defer_pool, \
                 tc.tile_pool(name="ppool", bufs=10) as ppool, \
                 tc.tile_pool(name="scpool", bufs=3) as scpool, \
                 tc.tile_pool(name="cupool", bufs=4) as cupool, \
                 tc.tile_pool(name="outp", bufs=3) as outp:

                fillers = []      # pending deferred-emission closures

                def push_projqk_B(m):
                    # reuses one deferred-psum slot: pq in the low half,
                    # pk in the high half
                    for n in range(s // NW):
                        state = {}

                        def mk_mm(which, k, n=n, state=state):
                            def emit():
                                if "t" not in state:
                                    state["t"] = defer_pool.tile(
                                        [P, max(d, 2 * NW)], f32,
                                        name="defer")
                                half = state["t"][:, 0:NW] if which == "q" \
                                    else state["t"][:, NW:2 * NW]
                                w_sb = wq_sb if which == "q" else wk_sb
                                nc.tensor.matmul(
                                    half, lhsT=w_sb[k][:, m * P:(m + 1) * P],
                                    rhs=xT_sb[k][:, n * NW:(n + 1) * NW],
                                    start=(k == 0), stop=(k == DC - 1))
                            return emit

                        def mk_fin(which, n=n, state=state):
                            def emit():
                                half = state["t"][:, 0:NW] if which == "q" \
                                    else state["t"][:, NW:2 * NW]
                                t_sb = qt_sb if which == "q" else kt_sb
                                b_sb = bq_sb if which == "q" else bk_sb
                                nc.vector.tensor_scalar_add(
                                    t_sb[m][:, n * NW:(n + 1) * NW], half,
                                    b_sb[:, m:m + 1])
                            return emit

                        for k in range(DC):
                            fillers.append(mk_mm("q", k))
                        fillers.append(mk_fin("q"))
                        for k in range(DC):
                            fillers.append(mk_mm("k", k))
                        fillers.append(mk_fin("k"))

                def oproj_mms(st, ops):
                    # contracts a head pair's 128 ctx dims in one matmul
                    out = []
                    for j in range(d // NW):
                        for p in range(HPC // 2):
                            def mm(j=j, p=p):
                                nc.tensor.matmul(
                                    ops()[:, j * NW:(j + 1) * NW],
                                    lhsT=ctxp_sb[p][:, st * P:(st + 1) * P],
                                    rhs=wop_sb[p][:, j * NW:(j + 1) * NW],
                                    start=(p == 0), stop=(p == HPC // 2 - 1),
                                    skip_group_check=True)
                            out.append(mm)
                    return out

                def push_oproj(st):
                    state = {}

                    def ops():
                        if "ops" not in state:
                            state["ops"] = defer_pool.tile(
                                [P, max(d, 2 * NW)], f32, name="defer")
                        return state["ops"]

                    def fin():
                        ot = outp.tile([P, d], f32, name="ot")
                        nc.vector.tensor_copy(ot[:], state["ops"][:, 0:d])
                        nc.sync.dma_start(part_d[st * P:(st + 1) * P, :], ot[:])

                    fillers.extend(oproj_mms(st, ops))
                    fillers.append(fin)

                def filler_step(n=1):
                    for _ in range(n):
                        if fillers:
                            fillers.pop(0)()

                def drain_fillers():
                    while fillers:
                        fillers.pop(0)()

                QH = min(512, s)       # per-head query half
                NQH = s // QH

                def pair_attn(mc, qh, per_tile=0, pre=None):
                    # Both heads of chunk mc process the SAME query half
                    # together.  Their score matmuls use PE row-strips 0-63 /
                    # 64-127 (tile_position auto-derived from base partition),
                    # so the two 64-deep matmuls run CONCURRENTLY in the
                    # sub-array grid: ~2x score throughput.  Head a occupies
                    # psum columns 0:QH, head b QH:2QH of shared tiles.
                    qs = slice(qh * QH, (qh + 1) * QH)
                    ca = slice(2 * mc * (HD + 1), (2 * mc + 1) * (HD + 1))
                    cb = slice((2 * mc + 1) * (HD + 1), (2 * mc + 2) * (HD + 1))
                    cps = pc_pool.tile([HD + 1, 2 * QH], f32, name="cps")
                    prev_pT = None
                    prev_k = -1
                    for k in range(KC):
                        if pre is not None and k < len(pre):
                            pre[k]()
                        sps = ps_pool.tile([P, 2 * QH], f32, name="sps")
                        nc.tensor.matmul(
                            sps[:, 0:QH],
                            lhsT=kt_sb[mc][0:HD, k * P:(k + 1) * P],
                            rhs=qt_sb[mc][0:HD, qs],
                            start=True, stop=True)
                        nc.tensor.matmul(
                            sps[:, QH:2 * QH],
                            lhsT=kt_sb[mc][HD:P, k * P:(k + 1) * P],
                            rhs=qt_sb[mc][HD:P, qs],
                            start=True, stop=True)
                        if prev_pT is not None:
                            nc.tensor.matmul(
                                cps[:, 0:QH], lhsT=va_sb[prev_k][:, ca],
                                rhs=prev_pT[:, 0:QH],
                                start=(prev_k == 0), stop=False,
                                skip_group_check=True)
                            nc.tensor.matmul(
                                cps[:, QH:2 * QH], lhsT=va_sb[prev_k][:, cb],
                                rhs=prev_pT[:, QH:2 * QH],
                                start=(prev_k == 0), stop=False,
                                skip_group_check=True)
                        pT = ppool.tile([P, 2 * QH], bf16, name="pT")
                        nc.scalar.activation(pT[:], sps[:], AF.Exp, scale=0.125)
                        prev_pT, prev_k = pT, k
                        filler_step(per_tile)
                    nc.tensor.matmul(
                        cps[:, 0:QH], lhsT=va_sb[prev_k][:, ca],
                        rhs=prev_pT[:, 0:QH], start=False, stop=True,
                        skip_group_check=True)
                    nc.tensor.matmul(
                        cps[:, QH:2 * QH], lhsT=va_sb[prev_k][:, cb],
                        rhs=prev_pT[:, QH:2 * QH], start=False, stop=True,
                        skip_group_check=True)
                    # evacuate + normalize both heads (odd head first so the
                    # tail of the whole kernel ends on the cheaper even path)
                    for par in (1, 0):
                        hh = 2 * mc + par
                        csl = slice(par * QH, par * QH + QH)
                        cu = cupool.tile([HD, QH], bf16, name="cu")
                        nc.vector.tensor_copy(cu[:], cps[0:HD, csl])
                        scb = scpool.tile([HD, QH], f32, name="scb")
                        nc.vector.reciprocal(scb[0:1, :],
                                             cps[HD:HD + 1, csl])
                        nc.gpsimd.partition_broadcast(scb[:], scb[0:1, :],
                                                      channels=HD)
                        if par == 0:
                            dst = ctxp_sb[mc][0:HD, qs]
                            nc.vector.tensor_tensor(dst, cu[:], scb[:],
                                                    ALU.mult)
                            nc.vector.tensor_scalar_add(
                                dst, dst, dock_sb[:, hh:hh + 1])
                        else:
                            # odd head: normalize at base 0, then DMA the 64
                            # partitions up into rows 64-127 of the pair tile
                            ctmp = cupool.tile([HD, QH], bf16, name="ctmp")
                            nc.vector.tensor_tensor(ctmp[:], cu[:], scb[:],
                                                    ALU.mult)
                            nc.vector.tensor_scalar_add(
                                ctmp[:], ctmp[:], dock_sb[:, hh:hh + 1])
                            nc.sync.dma_start(ctxp_sb[mc][HD:P, qs], ctmp[:])

                # V-projection groups are emitted inside the first pair's
                # first key loop (one seq tile per key tile, just ahead of the
                # ctx matmul that consumes it); Q/K m=1 projections drip
                # through the rest of pair 0.
                def mk_vgroup(st):
                    def emit():
                        pv = defer_pool.tile([P, max(d, 2 * NW)], f32,
                                             name="defer")[:, 0:DHC]
                        for k in range(DC):
                            nc.tensor.matmul(
                                pv[:], lhsT=xT_sb[k][:, st * P:(st + 1) * P],
                                rhs=wv_sb[k][:], start=(k == 0),
                                stop=(k == DC - 1), skip_group_check=True)
                        dst = va_sb[st][:].rearrange(
                            "p (h c) -> p h c", c=HD + 1)[:, :, 0:HD]
                        nc.vector.tensor_tensor(
                            dst, pv[:].rearrange("p (h c) -> p h c", c=HD),
                            bv_bc[:].rearrange("p (h c) -> p h c", c=HD),
                            ALU.add)
                    return emit

                vwork = [mk_vgroup(st) for st in range(ST)]
                pair_attn(0, 0, pre=vwork)   # ST == KC: all V inside
                if DHC // P > 1:
                    push_projqk_B(1)
                for qh in range(1, NQH):
                    pair_attn(0, qh, per_tile=2)
                drain_fillers()   # pair 1 needs qt/kt m=1 complete
                for qh in range(NQH):
                    pair_attn(1, qh)
                # O-projection: emitted last (lowest priority); each seq tile
                # becomes ready as soon as both pairs finish its query half,
                # so the scheduler weaves these into pair 1's PE stalls.
                # The last query half stays in the pipelined tail scope.
                for st in range(max(0, ST - QH // P)):
                    ops = defer_pool.tile([P, max(d, 2 * NW)], f32,
                                          name="defer")
                    for mm in oproj_mms(st, lambda ops=ops: ops):
                        mm()
                    ot = outp.tile([P, d], f32, name="ot")
                    nc.vector.tensor_copy(ot[:], ops[:, 0:d])
                    nc.sync.dma_start(part_d[st * P:(st + 1) * P, :], ot[:])

            # ---- O-projection tail for the last query chunk (pipelined) ----
            with tc.tile_pool(name="psum_o2", bufs=3, space="PSUM") as po2, \
                 tc.tile_pool(name="outp2", bufs=3) as outp2:
                for st in range(max(0, ST - (min(512, s) // P)), ST):
                    ops2 = po2.tile([P, d], f32, name="ops2")
                    for mm in oproj_mms(st, lambda: ops2):
                        mm()
                    ot2 = outp2.tile([P, d], f32, name="ot2")
                    nc.vector.tensor_copy(ot2[:], ops2[:])
                    nc.sync.dma_start(part_d[st * P:(st + 1) * P, :], ot2[:])

    nc.compile()
    return nc


_CACHE = {}


def _get_module():
    if "nc" not in _CACHE:
        _CACHE["nc"] = build_module()
    return _CACHE["nc"]


def _shard_inputs(x, docking_scores, Wq, bq, Wk, bk, Wv, bv, Wo, bo, beta):
    """Build the 8 per-core input maps. Returns (in_maps, omb_eff)."""
    x = np.asarray(x, np.float32)
    ds = np.asarray(docking_scores, np.float32)
    Wq = np.asarray(Wq, np.float32)
    Wk = np.asarray(Wk, np.float32)
    Wv = np.asarray(Wv, np.float32)
    Wo = np.asarray(Wo, np.float32)
    bq = np.asarray(bq, np.float32)
    bk = np.asarray(bk, np.float32)
    bv = np.asarray(bv, np.float32)
    beta = float(np.asarray(beta))
    omb = 1.0 - beta
    # guard the degenerate beta == 1 case: softmax part vanishes
    omb_eff = omb if abs(omb) > 1e-30 else 1e-30
    in_maps = []
    for c in range(NCORES):
        b = c // GROUPS
        g = c % GROUPS
        cols = slice(g * DHC, (g + 1) * DHC)
        in_maps.append({
            "xT": np.ascontiguousarray(x[b].T).astype(ml_dtypes.bfloat16),
            "wq": np.ascontiguousarray(Wq[:, cols]).astype(ml_dtypes.bfloat16),
            "wk": np.ascontiguousarray(Wk[:, cols]).astype(ml_dtypes.bfloat16),
            "wv": np.ascontiguousarray(Wv[:, cols]).astype(ml_dtypes.bfloat16),
            "wo": np.ascontiguousarray(Wo[cols, :]).astype(ml_dtypes.bfloat16),
            "bq": np.ascontiguousarray(bq[cols]),
            "bk": np.ascontiguousarray(bk[cols]),
            "bv": np.ascontiguousarray(bv[cols]),
            # dock_h = V_h^T @ (beta/(1-beta) ds) = ((x^T dsp) Wv + sum(dsp) bv)_h
            "dock": ((x[b].T @ (ds[b] * (beta / omb_eff))) @ Wv[:, cols]
                     + float((ds[b] * (beta / omb_eff)).sum())
                     * bv[cols]).astype(np.float32),
        })
    return in_maps, omb_eff


def kernel(x, docking_scores, Wq, bq, Wk, bk, Wv, bv, Wo, bo, beta):
    from concourse.bass_utils import run_bass_kernel_spmd

    nc = _get_module()
    in_maps, omb_eff = _shard_inputs(x, docking_scores, Wq, bq, Wk, bk,
                                     Wv, bv, Wo, bo, beta)
    res = run_bass_kernel_spmd(nc, in_maps, core_ids=list(range(NCORES)))
    bo = np.asarray(bo, np.float32)
    out = np.zeros((B, S, D), np.float32)
    for c in range(NCORES):
        out[c // GROUPS] += res.results[c]["part"]
    out = omb_eff * out + bo
    return out.astype(np.float32)


# ---------------------------------------------------------------------------
# reference math on numpy (for self tests only; mirrors reference.py)
def _numpy_ref(x, ds, Wq, bq, Wk, bk, Wv, bv, Wo, bo, beta, h=H):
    b, s, dd = x.shape
    hd = dd // h

    def heads(y):
        return y.reshape(b, s, h, hd).transpose(0, 2, 1, 3)

    Q = heads(x @ Wq + bq)
    K = heads(x @ Wk + bk)
    V = heads(x @ Wv + bv)
    sc = np.einsum("bhqd,bhkd->bhqk", Q, K) / np.float32(np.sqrt(hd))
    sc = sc - sc.max(axis=-1, keepdims=True)
    e = np.exp(sc)
    attn = e / e.sum(axis=-1, keepdims=True)
    attn = (1.0 - beta) * attn + beta * ds[:, None, None, :]
    ctx = np.einsum("bhqk,bhkd->bhqd", attn, V)
    ctx = ctx.transpose(0, 2, 1, 3).reshape(b, s, dd)
    return ctx @ Wo + bo


def _selftest_sim():
    """Small-shape functional check on CoreSim (no hardware)."""
    from concourse.bass_interp import CoreSim

    s, d = 256, 512
    nc = build_module(s=s, d=d, qchunk=256)
    rng = np.random.default_rng(0)
    x = rng.standard_normal((1, s, d), dtype=np.float32)
    ds = rng.random((1, s), dtype=np.float32)
    sc = 0.02
    h_small = d // HD  # heads in the small config
    Wq = rng.standard_normal((d, d), dtype=np.float32) * sc
    Wk = rng.standard_normal((d, d), dtype=np.float32) * sc
    Wv = rng.standard_normal((d, d), dtype=np.float32) * sc
    Wo = rng.standard_normal((d, d), dtype=np.float32) * sc
    bq = rng.standard_normal(d).astype(np.float32) * 0.1
    bk = rng.standard_normal(d).astype(np.float32) * 0.1
    bv = rng.standard_normal(d).astype(np.float32) * 0.1
    bo = np.zeros(d, np.float32)
    beta = 0.5
    omb = 1.0 - beta

    cols = slice(0, DHC)  # first 4 heads
    sim = CoreSim(nc)
    sim.tensor("xT")[:] = x[0].T
    sim.tensor("wq")[:] = Wq[:, cols]
    sim.tensor("wk")[:] = Wk[:, cols]
    sim.tensor("wv")[:] = Wv[:, cols]
    sim.tensor("wo")[:] = Wo[cols, :]
    sim.tensor("bq")[:] = bq[cols]
    sim.tensor("bk")[:] = bk[cols]
    sim.tensor("bv")[:] = bv[cols]
    dsp = ds[0] * (beta / omb)
    sim.tensor("dock")[:] = (x[0].T @ dsp) @ Wv[:, cols] + dsp.sum() * bv[cols]
    sim.simulate()
    part = sim.tensor("part").copy()

    # expected partial: heads 0..3 contribution, pre-(1-beta), no bo
    ref = _numpy_ref(x, ds, Wq, bq, Wk, bk, Wv, bv, Wo, bo, beta, h=h_small)
    # isolate first-4-heads partial by zeroing other head rows of Wo
    Wo_m = np.zeros_like(Wo)
    Wo_m[cols, :] = Wo[cols, :]
    ref_part = _numpy_ref(x, ds, Wq, bq, Wk, bk, Wv, bv, Wo_m, bo, beta,
                          h=h_small)
    got = omb * part
    err = np.abs(got - ref_part).max() / (np.abs(ref_part).max() + 1e-9)
    print("selftest sim rel err (first 4 heads partial):", err)
    assert err < 3e-2, err
    print("SELFTEST PASS")


def _timeline():
    """Cost-model timing estimate of the full-size per-core program."""
    from concourse.timeline_sim import TimelineSim

    nc = _get_module()
    tl = TimelineSim(nc, trace=False)
    t = tl.simulate()
    print(f"TimelineSim estimate: {t:.0f} ns")


if __name__ == "__main__":
    mode = sys.argv[1] if len(sys.argv) > 1 else "sim"
    if mode == "sim":
        _selftest_sim()
    elif mode == "timeline":
        _timeline()
